# revision 9
# baseline (speedup 1.0000x reference)
"""CorrelationDimensionLoss kernel for 8x Trainium2 NeuronCores (Bass, raw engine programming).

Math: reference computes S_m = sum_{i<j} sigmoid(K*(r_m - d_ij)) / cnt for 16
log-spaced thresholds r_m, then -slope of lstsq(log r, log S).

Device strategy (identical SPMD program on 8 cores, different data):
  - The 8192x8192 pairwise-distance matrix is covered by its 8x8 grid of
    1024x1024 blocks; only the upper triangle incl. diagonal (36 blocks = 72
    chunks of 1024x512) is computed. Each core gets 9 chunks: its 2
    diagonal-block chunks (counted x0.5 on host, self-pairs masked to d~3e4)
    + 7 off-diagonal chunks (x1).
  - Per chunk: PE computes d^2 via one K=34 augmented matmul per 128-row tile
    ([-2x_i, |x_i|^2, 1].[x_j, 1, |x_j|^2]); DVE clamps to >=0 into SBUF and
    adds +1e9 on self-pair diagonals; ACT takes sqrt -> d.
  - Per super-iteration (2 chunks = [128, 8192] d tile), threshold sums are
    split across engines, each sum landing in an accumulator column:
      ACT:  E' = exp(-10(d-5)) with fused accum -> T = sum E' serves all tail
            thresholds (r_m <= d_min-0.40) via sigmoid(z) = e^z - e^{2z} + O(e^{3z});
            plus one fused-accum sigmoid pass per "mid" threshold.
      DVE:  T2 = sum E'^2 (one tensor_tensor_reduce);
            the 2 largest eligible mids via sigmoid = E'/(E'+a_m):
            tensor_scalar_add -> reciprocal_approx_fast (~51 ULP) ->
            tensor_tensor_reduce(E' * y), scratching in the dead d^2 buffer.
    Thresholds with 10*(r_m - d_max) >= 18 saturate to exactly 1.0 in fp32
    (as in the reference): S_m = cnt with no device work.
  - Host gathers [128, n_cols*5] accumulators from 8 cores, reduces in fp64,
    assembles the 16 sums, tiny lstsq.
"""

import os
import numpy as np

import concourse.bass as bass
import concourse.mybir as mybir
from concourse.bass_utils import run_bass_kernel_spmd

N = 8192
D = 32
NC = 8
KSHARP = 10.0
BLK = 1024
CHW = 512
NCHUNK = 9
SUP_CHUNKS = [2, 2, 2, 2, 1]
EXP_SHIFT = 5.0     # E' = e^{-K(d-EXP_SHIFT)}: keeps T, T2=sum E'^2 and the
                    # DVE denominators E'+a_m inside normal fp32 for d in [2, 14]
MASK_BIG = 1e9
TAIL_MARGIN = 0.40  # r_m <= d_min - margin -> 2-term expansion, rel err <= e^{-20*margin} ~ 3e-4
SAT_Z = 18.0        # 10*(r_m - d_max) >= SAT_Z -> sigmoid == 1.0f exactly
A_MIN = 1e-37       # DVE-path a_m = e^{-K(r_m-EXP_SHIFT)} must stay normal fp32
N_DVE = 0           # DVE sigmoid offload disabled: this walrus build rejects the
                    # raw-ISA tensor_tensor_reduce / custom-DVE ops ("ISA wrong length")

_cache = {}

# exported for test.py
last_results = None
last_in_maps = None


def _chunk_assignment():
    offdiag = []
    for i in range(NC):
        for j in range(i + 1, NC):
            for h in range(2):
                offdiag.append((i, 2 * j + h))
    assert len(offdiag) == 56
    return [[(c, 2 * c), (c, 2 * c + 1)] + offdiag[7 * c:7 * c + 7] for c in range(NC)]


def _build_program(n_act, dve_a, repeat=1):
    """n_act: #ACT-sigmoid mids; dve_a: list of a_m consts for DVE-path mids.
    Accumulator columns per super-iter: [T, T2, act mids..., dve mids...]."""
    n_dve = len(dve_a)
    n_cols = 2 + n_act + n_dve
    outc = n_cols * len(SUP_CHUNKS)
    nbias = 1 + n_act  # E' bias + act-mid biases
    f32 = mybir.dt.float32
    AF = mybir.ActivationFunctionType
    ALU = mybir.AluOpType

    nc = bass.Bass("TRN2", target_bir_lowering=False, debug=False)
    rows_d = nc.dram_tensor("rows", [D + 2, NCHUNK * BLK], f32, kind="ExternalInput").ap()
    cols_d = nc.dram_tensor("cols", [D + 2, NCHUNK * CHW], f32, kind="ExternalInput").ap()
    mask_d = nc.dram_tensor("mask", [128, 128], f32, kind="ExternalInput").ap()
    bias_d = nc.dram_tensor("bias", [128, nbias], f32, kind="ExternalInput").ap()
    out_d = nc.dram_tensor("out", [128, outc], f32, kind="ExternalOutput").ap()

    W2 = 2 * 8 * CHW  # 8192

    from contextlib import ExitStack
    with ExitStack() as ctx:
        rows = ctx.enter_context(nc.sbuf_tensor("rows_sb", [D + 2, NCHUNK * BLK], f32)).ap()
        cols = ctx.enter_context(nc.sbuf_tensor("cols_sb", [D + 2, NCHUNK * CHW], f32)).ap()
        mask = ctx.enter_context(nc.sbuf_tensor("mask_sb", [128, 128], f32)).ap()
        bias = ctx.enter_context(nc.sbuf_tensor("bias_sb", [128, nbias], f32)).ap()
        d2 = ctx.enter_context(nc.sbuf_tensor("d2_sb", [128, W2], f32)).ap()
        dd = ctx.enter_context(nc.sbuf_tensor("d_sb", [128, W2], f32)).ap()
        esb = ctx.enter_context(nc.sbuf_tensor("e_sb", [128, W2], f32)).ap()
        scr = ctx.enter_context(nc.sbuf_tensor("scr_sb", [128, W2], mybir.dt.bfloat16)).ap()
        dum = ctx.enter_context(nc.sbuf_tensor("dum_sb", [128, 1], f32)).ap()
        acc = ctx.enter_context(nc.sbuf_tensor("acc_sb", [128, outc], f32)).ap()
        psum = [ctx.enter_context(nc.psum_tensor(f"ps{i}", [128, CHW], f32)).ap() for i in range(8)]
        dma_sem = ctx.enter_context(nc.semaphore("dma_sem"))
        pe_sem = ctx.enter_context(nc.semaphore("pe_sem"))
        dve_sem = ctx.enter_context(nc.semaphore("dve_sem"))
        sqrt_sem = ctx.enter_context(nc.semaphore("sqrt_sem"))
        e_sem = ctx.enter_context(nc.semaphore("e_sem"))
        sig_sem = ctx.enter_context(nc.semaphore("sig_sem"))
        done_sem = ctx.enter_context(nc.semaphore("done_sem"))
        block = ctx.enter_context(nc.Block())

        @block.gpsimd
        def _(g):
            g.dma_start(out=rows, in_=rows_d).then_inc(dma_sem, 16)
            g.dma_start(out=cols, in_=cols_d).then_inc(dma_sem, 16)
            g.dma_start(out=mask, in_=mask_d).then_inc(dma_sem, 16)
            g.dma_start(out=bias, in_=bias_d).then_inc(dma_sem, 16)
            g.wait_ge(done_sem, 1)
            g.dma_start(out=out_d, in_=acc).then_inc(dma_sem, 16)

        @block.tensor
        def _(t):
            t.wait_ge(dma_sem, 64)
            kg = 0
            for it in range(repeat):
                for k in range(NCHUNK):
                    if kg > 0:
                        t.wait_ge(dve_sem, kg)  # prior chunk drained from PSUM
                    mm = None
                    for ti in range(8):
                        mm = t.matmul(
                            psum[ti],
                            lhsT=rows[:, BLK * k + 128 * ti: BLK * k + 128 * (ti + 1)],
                            rhs=cols[:, CHW * k: CHW * (k + 1)],
                            start=True, stop=True,
                        )
                    mm.then_inc(pe_sem, 1)
                    kg += 1

        @block.vector
        def _(v):
            kg = 0
            for it in range(repeat):
                for s, nch in enumerate(SUP_CHUNKS):
                    S = len(SUP_CHUNKS) * it + s
                    W = 4096 * nch
                    # --- drain this super-iter's chunks PSUM -> d2 ---
                    for ci in range(nch):
                        v.wait_ge(pe_sem, kg + 1)
                        if ci == 0 and S >= 1:
                            v.wait_ge(sqrt_sem, S)  # d2 free: sqrt of prev super-iter done
                        base = 4096 * ci
                        for ti in range(8):
                            op = v.tensor_scalar_max(
                                d2[:, base + CHW * ti: base + CHW * (ti + 1)], psum[ti], 0.0)
                        if s == 0:
                            tis = range(0, 4) if ci == 0 else range(4, 8)
                            for ti in tis:
                                off = base + CHW * ti + 128 * (ti if ci == 0 else ti - 4)
                                op = v.tensor_add(d2[:, off:off + 128], d2[:, off:off + 128], mask)
                        op.then_inc(dve_sem, 1)
                        kg += 1
                    # --- threshold work reading E'(s): T2 + dve mids ---
                    # (d2 is dead as d^2 once sqrt(s) ran; reuse it as scratch)
                    v.wait_ge(e_sem, S + 1)
                    col = s * n_cols
                    assert not dve_a, "DVE sigmoid path unsupported by this walrus"
                    op = v.scalar_tensor_tensor(
                        dum.broadcast_to((128, W)), esb[:, :W], 1.0, esb[:, :W],
                        ALU.mult, ALU.mult, accum_out=acc[:, col + 1:col + 2])
                    op.then_inc(sig_sem, 1)

        @block.scalar
        def _(sc):
            cum = 0
            for it in range(repeat):
                for s, nch in enumerate(SUP_CHUNKS):
                    S = len(SUP_CHUNKS) * it + s
                    cum += nch
                    W = 4096 * nch
                    col = s * n_cols
                    sc.wait_ge(dve_sem, cum)
                    op = sc.activation(dd[:, :W], d2[:, :W], AF.Sqrt)
                    op.then_inc(sqrt_sem, 1)
                    if S >= 1:
                        sc.wait_ge(sig_sem, S)  # e_sb free: DVE done reading E'(S-1)
                    sc.activation(esb[:, :W], dd[:, :W], AF.Exp, scale=-KSHARP,
                                  bias=bias[:, 0:1],
                                  accum_out=acc[:, col:col + 1]).then_inc(e_sem, 1)
                    last = None
                    for i in range(n_act):
                        last = sc.activation(scr[:, :W], dd[:, :W], AF.Sigmoid, scale=-KSHARP,
                                             bias=bias[:, 1 + i:2 + i],
                                             accum_out=acc[:, col + 2 + i:col + 3 + i])
                    if s == len(SUP_CHUNKS) - 1 and it == repeat - 1:
                        (last if last is not None else op).then_inc(done_sem, 1)
    return nc


def _dist_extremes(pts):
    sq = np.einsum("ij,ij->i", pts, pts)
    dmin, dmax = np.inf, 0.0
    B = 1024
    for i0 in range(0, N, B):
        g = pts[i0:i0 + B] @ pts.T
        d2b = sq[i0:i0 + B, None] + sq[None, :] - 2.0 * g
        for r in range(d2b.shape[0]):
            d2b[r, i0 + r] = np.inf
        dmin = min(dmin, float(np.sqrt(max(d2b.min(), 0.0))))
        for r in range(d2b.shape[0]):
            d2b[r, i0 + r] = 0.0
        dmax = max(dmax, float(np.sqrt(max(d2b.max(), 0.0))))
    return dmin, dmax


def kernel(points, r_values):
    global last_results, last_in_maps
    points = np.ascontiguousarray(np.asarray(points, dtype=np.float32))
    r_values = np.asarray(r_values, dtype=np.float32)
    assert points.shape == (N, D) and r_values.shape == (16,)
    rv = r_values.astype(np.float64)
    nr = len(rv)

    dmin, dmax = _dist_extremes(points)

    tail = [m for m in range(nr) if rv[m] <= dmin - TAIL_MARGIN]
    sat = [m for m in range(nr) if KSHARP * (rv[m] - dmax) >= SAT_Z]
    mid = [m for m in range(nr) if m not in tail and m not in sat]
    # DVE-path eligibility: a_m stays normal fp32
    elig = [m for m in mid if np.exp(-KSHARP * (rv[m] - EXP_SHIFT)) >= A_MIN]
    dve_mids = sorted(elig, key=lambda m: -rv[m])[:N_DVE]
    act_mids = [m for m in mid if m not in dve_mids]
    n_act, n_dve = len(act_mids), len(dve_mids)
    dve_a = [float(np.exp(-KSHARP * (rv[m] - EXP_SHIFT))) for m in dve_mids]
    n_cols = 2 + n_act + n_dve

    key = (n_act, tuple(np.float32(dve_a)))
    if key not in _cache:
        _cache[key] = _build_program(n_act, dve_a)
    nc = _cache[key]

    sq = np.einsum("ij,ij->i", points, points).astype(np.float32)
    ones = np.ones(N, dtype=np.float32)
    A = np.concatenate([(-2.0 * points).T, sq[None, :], ones[None, :]], axis=0)
    B = np.concatenate([points.T, ones[None, :], sq[None, :]], axis=0)

    assign = _chunk_assignment()
    maskarr = MASK_BIG * np.eye(128, dtype=np.float32)
    biasarr = np.zeros((128, 1 + n_act), dtype=np.float32)
    biasarr[:, 0] = KSHARP * EXP_SHIFT
    for i, m in enumerate(act_mids):
        biasarr[:, 1 + i] = KSHARP * r_values[m]
    in_maps = []
    for c in range(NC):
        rows = np.empty((D + 2, NCHUNK * BLK), dtype=np.float32)
        colsb = np.empty((D + 2, NCHUNK * CHW), dtype=np.float32)
        for k, (rb, ch) in enumerate(assign[c]):
            rows[:, k * BLK:(k + 1) * BLK] = A[:, rb * BLK:(rb + 1) * BLK]
            colsb[:, k * CHW:(k + 1) * CHW] = B[:, ch * CHW:(ch + 1) * CHW]
        in_maps.append({"rows": rows, "cols": colsb, "mask": maskarr, "bias": biasarr})
    last_in_maps = in_maps

    trace = bool(os.environ.get("CDL_TRACE"))
    res = run_bass_kernel_spmd(nc, in_maps, core_ids=list(range(NC)), trace=trace)
    last_results = res

    totals = np.zeros(n_cols, dtype=np.float64)
    for c in range(NC):
        accm = res.results[c]["out"].astype(np.float64)
        for s in range(len(SUP_CHUNKS)):
            w = 0.5 if s == 0 else 1.0
            totals += w * accm[:, s * n_cols:(s + 1) * n_cols].sum(axis=0)

    cnt = N * (N - 1) / 2.0
    S = np.zeros(nr, dtype=np.float64)
    T1, T2 = totals[0], totals[1]
    for m in tail:
        S[m] = (np.exp(KSHARP * (rv[m] - EXP_SHIFT)) * T1
                - np.exp(2.0 * KSHARP * (rv[m] - EXP_SHIFT)) * T2)
    for i, m in enumerate(act_mids):
        S[m] = totals[2 + i]
    for j, m in enumerate(dve_mids):
        S[m] = totals[2 + n_act + j]
    for m in sat:
        S[m] = cnt

    corr = S / cnt
    logr = np.log(rv)
    logc = np.log(corr)
    Amat = np.stack([logr, np.ones_like(logr)], axis=1)
    sol = np.linalg.solve(Amat.T @ Amat, Amat.T @ logc)
    return np.asarray(-sol[0], dtype=np.float32)


# revision 13
# speedup vs baseline: 4.8397x; 4.8397x over previous
"""CorrelationDimensionLoss kernel for 8x Trainium2 NeuronCores (Bass, raw engine programming).

Math: reference computes S_m = sum_{i<j} sigmoid(K*(r_m - d_ij)) / cnt for 16
log-spaced thresholds r_m, then -slope of lstsq(log r, log S).

Device strategy (identical SPMD program on 8 cores, different data):
  - The 8192x8192 pairwise-distance matrix is covered by its 8x8 grid of
    1024x1024 blocks; only the upper triangle incl. diagonal (36 blocks = 72
    chunks of 1024x512) is computed. Each core gets 9 chunks: its 2
    diagonal-block chunks (counted x0.5 on host, self-pairs masked to d~3e4)
    + 7 off-diagonal chunks (x1).
  - Per chunk: PE computes d^2 via one K=34 augmented matmul per 128-row tile
    ([-2x_i, |x_i|^2, 1].[x_j, 1, |x_j|^2]); DVE clamps to >=0 into SBUF and
    adds +1e9 on self-pair diagonals; ACT takes sqrt -> d.
  - Per super-iteration (2 chunks = [128, 8192] d tile), threshold sums are
    split across engines, each sum landing in an accumulator column:
      ACT:  E' = exp(-10(d-5)) with fused accum -> T = sum E' serves all tail
            thresholds (r_m <= d_min-0.40) via sigmoid(z) = e^z - e^{2z} + O(e^{3z});
            plus one fused-accum sigmoid pass per "mid" threshold.
      DVE:  T2 = sum E'^2 (one tensor_tensor_reduce);
            the 2 largest eligible mids via sigmoid = E'/(E'+a_m):
            tensor_scalar_add -> reciprocal_approx_fast (~51 ULP) ->
            tensor_tensor_reduce(E' * y), scratching in the dead d^2 buffer.
    Thresholds with 10*(r_m - d_max) >= 18 saturate to exactly 1.0 in fp32
    (as in the reference): S_m = cnt with no device work.
  - Host gathers [128, n_cols*5] accumulators from 8 cores, reduces in fp64,
    assembles the 16 sums, tiny lstsq.
"""

import os
import numpy as np

import concourse.bass as bass
import concourse.mybir as mybir
from concourse.bass_utils import run_bass_kernel_spmd

N = 8192
D = 32
NC = 8
KSHARP = 10.0
BLK = 1024
CHW = 512
NCHUNK = 9
SUP_CHUNKS = [2, 2, 2, 2, 1]
EXP_SHIFT = 5.0     # E' = e^{-K(d-EXP_SHIFT)}: keeps T, T2=sum E'^2 and the
                    # DVE denominators E'+a_m inside normal fp32 for d in [2, 14]
MASK_BIG = 1e9
TAIL_MARGIN = 0.40  # r_m <= d_min - margin -> 2-term expansion, rel err <= e^{-20*margin} ~ 3e-4
SAT_Z = 18.0        # 10*(r_m - d_max) >= SAT_Z -> sigmoid == 1.0f exactly
A_MIN = 1e-37       # DVE-path a_m = e^{-K(r_m-EXP_SHIFT)} must stay normal fp32
N_DVE = 0           # DVE sigmoid offload disabled: this walrus build rejects the
                    # raw-ISA tensor_tensor_reduce / custom-DVE ops ("ISA wrong length")

_cache = {}


def _chunk_tiles(k):
    # chunk 0 (first diagonal-block half) keeps only its 4 diagonal-crossing
    # tiles; its 4 fully-below-diagonal tiles are never computed.
    return range(4) if k == 0 else range(8)


def _chunk_width(k):
    return len(_chunk_tiles(k)) * CHW


def _sup_layout(s):
    """chunks of super-iter s and their packed d2 column bases"""
    ch = [2 * s, 2 * s + 1] if s < len(SUP_CHUNKS) - 1 else [2 * s]
    bases = [0]
    for k in ch[:-1]:
        bases.append(bases[-1] + _chunk_width(k))
    return ch, bases


# exported for test.py
last_results = None
last_in_maps = None


def _chunk_assignment():
    offdiag = []
    for i in range(NC):
        for j in range(i + 1, NC):
            for h in range(2):
                offdiag.append((i, 2 * j + h))
    assert len(offdiag) == 56
    return [[(c, 2 * c), (c, 2 * c + 1)] + offdiag[7 * c:7 * c + 7] for c in range(NC)]


def _build_program(n_act, dve_a, repeat=1, split_dma=True, use_memset=True):
    """n_act: #ACT-sigmoid mids; dve_a: list of a_m consts for DVE-path mids.
    Accumulator columns per super-iter: [T, T2, act mids..., dve mids...]."""
    n_dve = len(dve_a)
    n_cols = 2 + n_act + n_dve
    outc = n_cols * len(SUP_CHUNKS)
    nbias = 1 + n_act  # E' bias + act-mid biases
    f32 = mybir.dt.float32
    AF = mybir.ActivationFunctionType
    ALU = mybir.AluOpType

    nc = bass.Bass("TRN2", target_bir_lowering=False, debug=False)
    rows_d = nc.dram_tensor("rows", [D + 2, NCHUNK * BLK], f32, kind="ExternalInput").ap()
    cols_d = nc.dram_tensor("cols", [D + 2, NCHUNK * CHW], f32, kind="ExternalInput").ap()
    mask_d = nc.dram_tensor("mask", [128, 128], f32, kind="ExternalInput").ap()
    bias_d = nc.dram_tensor("bias", [128, nbias], f32, kind="ExternalInput").ap()
    out_d = nc.dram_tensor("out", [128, outc], f32, kind="ExternalOutput").ap()

    W2 = 2 * 8 * CHW  # 8192
    # DMA completions are out of order across queues; a plain >= count can be
    # satisfied by the small mask/bias transfers before rows/cols land. Every
    # engine therefore waits for ALL input DMAs.
    ALL_DONE = (8 if split_dma else 4) * 16

    from contextlib import ExitStack
    with ExitStack() as ctx:
        rows = ctx.enter_context(nc.sbuf_tensor("rows_sb", [D + 2, NCHUNK * BLK], f32)).ap()
        cols = ctx.enter_context(nc.sbuf_tensor("cols_sb", [D + 2, NCHUNK * CHW], f32)).ap()
        mask = ctx.enter_context(nc.sbuf_tensor("mask_sb", [128, 128], f32)).ap()
        bias = ctx.enter_context(nc.sbuf_tensor("bias_sb", [128, nbias], f32)).ap()
        d2 = ctx.enter_context(nc.sbuf_tensor("d2_sb", [128, W2], f32)).ap()
        dd = ctx.enter_context(nc.sbuf_tensor("d_sb", [128, W2], f32)).ap()
        esb = ctx.enter_context(nc.sbuf_tensor("e_sb", [128, W2], f32)).ap()
        scr = ctx.enter_context(nc.sbuf_tensor("scr_sb", [128, W2], mybir.dt.bfloat16)).ap()
        dum = ctx.enter_context(nc.sbuf_tensor("dum_sb", [128, 1], f32)).ap()
        acc = ctx.enter_context(nc.sbuf_tensor("acc_sb", [128, outc], f32)).ap()
        psum = [ctx.enter_context(nc.psum_tensor(f"ps{i}", [128, CHW], f32)).ap() for i in range(8)]
        dma_sem = ctx.enter_context(nc.semaphore("dma_sem"))
        pe_sem = ctx.enter_context(nc.semaphore("pe_sem"))
        dve_sem = ctx.enter_context(nc.semaphore("dve_sem"))
        sqrt_sem = ctx.enter_context(nc.semaphore("sqrt_sem"))
        e_sem = ctx.enter_context(nc.semaphore("e_sem"))
        sig_sem = ctx.enter_context(nc.semaphore("sig_sem"))
        done_sem = ctx.enter_context(nc.semaphore("done_sem"))
        block = ctx.enter_context(nc.Block())

        @block.gpsimd
        def _(g):
            if split_dma:
                RQ = NCHUNK * BLK // 4
                for q in range(4):
                    g.dma_start(out=rows[:, RQ * q:RQ * (q + 1)],
                                in_=rows_d[:, RQ * q:RQ * (q + 1)]).then_inc(dma_sem, 16)
                CQ = NCHUNK * CHW // 2
                for q in range(2):
                    g.dma_start(out=cols[:, CQ * q:CQ * (q + 1)],
                                in_=cols_d[:, CQ * q:CQ * (q + 1)]).then_inc(dma_sem, 16)
            else:
                g.dma_start(out=rows, in_=rows_d).then_inc(dma_sem, 16)
                g.dma_start(out=cols, in_=cols_d).then_inc(dma_sem, 16)
            g.dma_start(out=mask, in_=mask_d).then_inc(dma_sem, 16)
            g.dma_start(out=bias, in_=bias_d).then_inc(dma_sem, 16)
            g.wait_ge(done_sem, 1)
            g.dma_start(out=out_d, in_=acc).then_inc(dma_sem, 16)

        @block.tensor
        def _(t):
            t.wait_ge(dma_sem, ALL_DONE)  # all input DMAs done
            kg = 0
            for it in range(repeat):
                for k in range(NCHUNK):
                    if kg > 0:
                        t.wait_ge(dve_sem, kg)  # prior chunk drained from PSUM
                    mm = None
                    for ti in _chunk_tiles(k):
                        mm = t.matmul(
                            psum[ti],
                            lhsT=rows[:, BLK * k + 128 * ti: BLK * k + 128 * (ti + 1)],
                            rhs=cols[:, CHW * k: CHW * (k + 1)],
                            start=True, stop=True,
                        )
                    mm.then_inc(pe_sem, 1)
                    kg += 1

        @block.vector
        def _(v):
            kg = 0
            for it in range(repeat):
                for s in range(len(SUP_CHUNKS)):
                    S = len(SUP_CHUNKS) * it + s
                    chunks, bases = _sup_layout(s)
                    W = sum(_chunk_width(k) for k in chunks)
                    # --- drain this super-iter's chunks PSUM -> d2 ---
                    for ci, k in enumerate(chunks):
                        v.wait_ge(pe_sem, kg + 1)
                        if ci == 0 and S >= 1:
                            v.wait_ge(sqrt_sem, S)  # d2 free: sqrt of prev super-iter done
                        if it == 0 and k == 0:
                            v.wait_ge(dma_sem, ALL_DONE)  # mask loaded
                        base = bases[ci]
                        for ti in _chunk_tiles(k):
                            op = v.tensor_scalar_max(
                                d2[:, base + CHW * ti: base + CHW * (ti + 1)], psum[ti], 0.0)
                        if s == 0:
                            # diagonal-crossing tiles: zero-out (d2 := BIG) the
                            # at-or-below-diagonal region so each unordered pair
                            # counts exactly once (weight 1 on host).
                            crossing = [(ti, ti) for ti in range(4)] if k == 2 * s else \
                                       [(ti, ti - 4) for ti in range(4, 8)]
                            for ti, tp in crossing:
                                tb = base + CHW * ti
                                if tp > 0:
                                    if use_memset:
                                        op = v.memset(d2[:, tb:tb + 128 * tp], MASK_BIG)
                                    else:
                                        op = v.tensor_scalar(d2[:, tb:tb + 128 * tp],
                                                             d2[:, tb:tb + 128 * tp],
                                                             0.0, MASK_BIG,
                                                             ALU.mult, ALU.add)
                                op = v.tensor_add(d2[:, tb + 128 * tp:tb + 128 * tp + 128],
                                                  d2[:, tb + 128 * tp:tb + 128 * tp + 128], mask)
                        op.then_inc(dve_sem, 1)
                        kg += 1
                    # --- threshold work reading E'(s): T2 + dve mids ---
                    # (d2 is dead as d^2 once sqrt(s) ran; reuse it as scratch)
                    v.wait_ge(e_sem, S + 1)
                    col = s * n_cols
                    assert not dve_a, "DVE sigmoid path unsupported by this walrus"
                    op = v.scalar_tensor_tensor(
                        dum.broadcast_to((128, W)), esb[:, :W], 1.0, esb[:, :W],
                        ALU.mult, ALU.mult, accum_out=acc[:, col + 1:col + 2])
                    op.then_inc(sig_sem, 1)

        @block.scalar
        def _(sc):
            cum = 0
            for it in range(repeat):
                for s in range(len(SUP_CHUNKS)):
                    S = len(SUP_CHUNKS) * it + s
                    chunks, _b = _sup_layout(s)
                    cum += len(chunks)
                    W = sum(_chunk_width(k) for k in chunks)
                    col = s * n_cols
                    if S == 0:
                        sc.wait_ge(dma_sem, ALL_DONE)  # bias loaded
                    sc.wait_ge(dve_sem, cum)
                    op = sc.activation(dd[:, :W], d2[:, :W], AF.Sqrt)
                    op.then_inc(sqrt_sem, 1)
                    if S >= 1:
                        sc.wait_ge(sig_sem, S)  # e_sb free: DVE done reading E'(S-1)
                    sc.activation(esb[:, :W], dd[:, :W], AF.Exp, scale=-KSHARP,
                                  bias=bias[:, 0:1],
                                  accum_out=acc[:, col:col + 1]).then_inc(e_sem, 1)
                    last = None
                    for i in range(n_act):
                        last = sc.activation(scr[:, :W], dd[:, :W], AF.Sigmoid, scale=-KSHARP,
                                             bias=bias[:, 1 + i:2 + i],
                                             accum_out=acc[:, col + 2 + i:col + 3 + i])
                    if s == len(SUP_CHUNKS) - 1 and it == repeat - 1:
                        (last if last is not None else op).then_inc(done_sem, 1)
    return nc


def _dist_extremes(pts):
    sq = np.einsum("ij,ij->i", pts, pts)
    dmin, dmax = np.inf, 0.0
    B = 1024
    for i0 in range(0, N, B):
        g = pts[i0:i0 + B] @ pts.T
        d2b = sq[i0:i0 + B, None] + sq[None, :] - 2.0 * g
        for r in range(d2b.shape[0]):
            d2b[r, i0 + r] = np.inf
        dmin = min(dmin, float(np.sqrt(max(d2b.min(), 0.0))))
        for r in range(d2b.shape[0]):
            d2b[r, i0 + r] = 0.0
        dmax = max(dmax, float(np.sqrt(max(d2b.max(), 0.0))))
    return dmin, dmax


def kernel(points, r_values):
    global last_results, last_in_maps
    points = np.ascontiguousarray(np.asarray(points, dtype=np.float32))
    r_values = np.asarray(r_values, dtype=np.float32)
    assert points.shape == (N, D) and r_values.shape == (16,)
    rv = r_values.astype(np.float64)
    nr = len(rv)

    dmin, dmax = _dist_extremes(points)

    tail = [m for m in range(nr) if rv[m] <= dmin - TAIL_MARGIN]
    sat = [m for m in range(nr) if KSHARP * (rv[m] - dmax) >= SAT_Z]
    mid = [m for m in range(nr) if m not in tail and m not in sat]
    # DVE-path eligibility: a_m stays normal fp32
    elig = [m for m in mid if np.exp(-KSHARP * (rv[m] - EXP_SHIFT)) >= A_MIN]
    dve_mids = sorted(elig, key=lambda m: -rv[m])[:N_DVE]
    act_mids = [m for m in mid if m not in dve_mids]
    n_act, n_dve = len(act_mids), len(dve_mids)
    dve_a = [float(np.exp(-KSHARP * (rv[m] - EXP_SHIFT))) for m in dve_mids]
    n_cols = 2 + n_act + n_dve

    key = (n_act, tuple(np.float32(dve_a)))
    if key not in _cache:
        _cache[key] = _build_program(n_act, dve_a)
    nc = _cache[key]

    sq = np.einsum("ij,ij->i", points, points).astype(np.float32)
    ones = np.ones(N, dtype=np.float32)
    A = np.concatenate([(-2.0 * points).T, sq[None, :], ones[None, :]], axis=0)
    B = np.concatenate([points.T, ones[None, :], sq[None, :]], axis=0)

    assign = _chunk_assignment()
    maskarr = MASK_BIG * np.tril(np.ones((128, 128), dtype=np.float32))
    biasarr = np.zeros((128, 1 + n_act), dtype=np.float32)
    biasarr[:, 0] = KSHARP * EXP_SHIFT
    for i, m in enumerate(act_mids):
        biasarr[:, 1 + i] = KSHARP * r_values[m]
    in_maps = []
    for c in range(NC):
        rows = np.empty((D + 2, NCHUNK * BLK), dtype=np.float32)
        colsb = np.empty((D + 2, NCHUNK * CHW), dtype=np.float32)
        for k, (rb, ch) in enumerate(assign[c]):
            rows[:, k * BLK:(k + 1) * BLK] = A[:, rb * BLK:(rb + 1) * BLK]
            colsb[:, k * CHW:(k + 1) * CHW] = B[:, ch * CHW:(ch + 1) * CHW]
        in_maps.append({"rows": rows, "cols": colsb, "mask": maskarr, "bias": biasarr})
    last_in_maps = in_maps

    trace = bool(os.environ.get("CDL_TRACE"))
    res = run_bass_kernel_spmd(nc, in_maps, core_ids=list(range(NC)), trace=trace)
    last_results = res

    totals = np.zeros(n_cols, dtype=np.float64)
    for c in range(NC):
        accm = res.results[c]["out"].astype(np.float64)
        for s in range(len(SUP_CHUNKS)):
            totals += accm[:, s * n_cols:(s + 1) * n_cols].sum(axis=0)

    cnt = N * (N - 1) / 2.0
    S = np.zeros(nr, dtype=np.float64)
    T1, T2 = totals[0], totals[1]
    for m in tail:
        S[m] = (np.exp(KSHARP * (rv[m] - EXP_SHIFT)) * T1
                - np.exp(2.0 * KSHARP * (rv[m] - EXP_SHIFT)) * T2)
    for i, m in enumerate(act_mids):
        S[m] = totals[2 + i]
    for j, m in enumerate(dve_mids):
        S[m] = totals[2 + n_act + j]
    for m in sat:
        S[m] = cnt

    corr = S / cnt
    logr = np.log(rv)
    logc = np.log(corr)
    Amat = np.stack([logr, np.ones_like(logr)], axis=1)
    sol = np.linalg.solve(Amat.T @ Amat, Amat.T @ logc)
    return np.asarray(-sol[0], dtype=np.float32)
